# revision 21
# baseline (speedup 1.0000x reference)
"""Causal self-attention (B=4, T=2048, C=1024, H=16) on 8 trn2 NeuronCores.

Sharding: tensor-parallel over heads. Core c owns heads {2c, 2c+1}:
  - computes Q,K,V projections for its 2 heads (full batch/sequence),
  - causal attention for its heads,
  - a partial output projection (row-slice of W_proj),
and the host sums the 8 partial projections (+ b_proj).

v2 changes vs baseline (705us):
  - All matmul operands in fp16 instead of float32r. fp32-HIGH matmuls
    trip the TRN2 power throttle (HAM shows util limited to 4/8 for 79%
    of the run); fp16 runs 1 cyc/row at full 2.4 GHz without throttling
    and is precise enough (rel err ~1e-3 << 2e-2 gate).
  - Attention processed in 512-wide q chunks; the two heads' S^T tiles
    live in one [128, 2, 512] PSUM tile so a single Exp activation
    covers both heads (halves scalar-engine instruction overhead).
  - Softmax denominators: reciprocal_approx_fast on the [1,512] l-row
    (the old full-tile nc.vector.reciprocal cost 106us of DVE time).
  - PSUM->SBUF evacuations (out-proj tiles, V transposes) and the
    causal mask add run on the idle Pool/GpSimd engine instead of DVE.
  - Output partials stored as fp16 (halves output DMA), summed on host.
  - x loaded with one DMA per 512-token tile instead of 8.
"""

import numpy as np

import concourse.bacc as bacc
import concourse.bass as bass
import concourse.tile as tile
from concourse import mybir
from concourse.bass_utils import run_bass_kernel_spmd
from concourse.masks import make_identity

F32 = mybir.dt.float32
F32R = mybir.dt.float32r
F16 = mybir.dt.float16
AF = mybir.ActivationFunctionType
ALU = mybir.AluOpType

N_CORES = 8
D_MODEL = 1024
N_HEADS = 16
HEAD_DIM = 64
H_LOC = 2            # heads per core
D_LOC = H_LOC * HEAD_DIM   # 128
SCALE = 1.0 / np.sqrt(HEAD_DIM)
NEG = -30000.0       # masked logits; exp(scale*NEG) == 0, fp16-safe


def build_program(B=4, T=2048):
    TOK = B * T
    TT = TOK // 512          # tok tiles of 512 for the QKV matmul
    CT = D_MODEL // 128      # contraction tiles
    NW = T // 512            # q-chunks per batch (512 wide)
    assert T % 512 == 0 and TOK % 512 == 0

    nc = bacc.Bacc(
        "TRN2", target_bir_lowering=False, debug=False, num_devices=N_CORES
    )
    xT = nc.dram_tensor("xT", [D_MODEL, TOK], F16, kind="ExternalInput").ap()
    wq = nc.dram_tensor("wq", [D_MODEL, D_LOC], F16, kind="ExternalInput").ap()
    wk = nc.dram_tensor("wk", [D_MODEL, D_LOC], F16, kind="ExternalInput").ap()
    wv = nc.dram_tensor("wv", [D_MODEL, D_LOC], F16, kind="ExternalInput").ap()
    bq = nc.dram_tensor("bq", [D_LOC, 1], F32, kind="ExternalInput").ap()
    bk = nc.dram_tensor("bk", [D_LOC, 1], F32, kind="ExternalInput").ap()
    bv = nc.dram_tensor("bv", [D_LOC, 1], F32, kind="ExternalInput").ap()
    wp = nc.dram_tensor("wp", [D_LOC, D_MODEL], F16, kind="ExternalInput").ap()
    outp = nc.dram_tensor("outp", [TOK, D_MODEL], F16, kind="ExternalOutput").ap()

    with tile.TileContext(nc) as tc:
        with (
            tc.tile_pool(name="const", bufs=1) as const,
            tc.tile_pool(name="res", bufs=1) as res,
        ):
            # --- constants -------------------------------------------------
            wq_sb = const.tile([128, CT, D_LOC], F16, tag="wq")
            wk_sb = const.tile([128, CT, D_LOC], F16, tag="wk")
            wv_sb = const.tile([128, CT, D_LOC], F16, tag="wv")
            for w_sb, w_dram in ((wq_sb, wq), (wk_sb, wk), (wv_sb, wv)):
                nc.sync.dma_start(
                    out=w_sb, in_=w_dram.rearrange("(ct p) d -> p ct d", p=128)
                )
            wp_sb = const.tile([128, D_MODEL], F16, tag="wp")
            nc.sync.dma_start(out=wp_sb, in_=wp)
            bq_sb = const.tile([128, 1], F32, tag="bq")
            bk_sb = const.tile([128, 1], F32, tag="bk")
            bv_sb = const.tile([128, 1], F32, tag="bv")
            for b_sb, b_dram in ((bq_sb, bq), (bk_sb, bk), (bv_sb, bv)):
                nc.sync.dma_start(out=b_sb, in_=b_dram)

            ones_f32 = const.tile([128, 128], F32, tag="ones_f32")
            nc.vector.memset(ones_f32, 1.0)
            ones_r = const.tile([128, 128], F32R, tag="ones_r")
            nc.vector.tensor_copy(ones_r, ones_f32)
            ident_f32 = const.tile([128, 128], F32, tag="ident_f32")
            make_identity(nc, ident_f32)
            ident = const.tile([128, 128], F16, tag="ident")
            nc.vector.tensor_copy(ident, ident_f32)

            # --- resident tensors -----------------------------------------
            qt_s = res.tile([128, TOK], F16, tag="qt")   # [d(2 heads), tok]
            kt_s = res.tile([128, TOK], F16, tag="kt")
            # token-major V with ones column: [tok(128), head, blk, 65]
            vtm = res.tile([128, H_LOC, TOK // 128, 65], F16, tag="vtm")
            ot_s = res.tile([128, TOK], F16, tag="ot")   # attention out, d-major
            nc.vector.tensor_copy(
                vtm[:, :, :, 64],
                ones_f32.rearrange("p (h b) -> p h b", h=H_LOC)[:, :, :TOK // 128],
            )

            # ================= phase 1: QKV projections ===================
            with (
                tc.tile_pool(name="xst", bufs=3) as xst,
                tc.tile_pool(name="vtt", bufs=3) as vtt,
                tc.tile_pool(name="ps1", bufs=1, space="PSUM") as ps1,
            ):
                for tt in range(TT):
                    t0 = tt * 512
                    xt = xst.tile([128, CT, 512], F16, tag="x")
                    # issue x loads on the (idle in phase 1) Activation HWDGE
                    # queue so they run in parallel with the weight loads on
                    # the SP queue.
                    nc.scalar.dma_start(
                        out=xt,
                        in_=xT.rearrange("(ct p) t -> p ct t", p=128)[
                            :, :, t0:t0 + 512
                        ],
                    )
                    pq = ps1.tile([128, 512], F32, tag="acc", bufs=6)
                    pk = ps1.tile([128, 512], F32, tag="acc", bufs=6)
                    pv = ps1.tile([128, 512], F32, tag="acc", bufs=6)
                    for ct in range(CT):
                        st, sp = ct == 0, ct == CT - 1
                        nc.tensor.matmul(
                            pq, (wq_sb[:, ct, :]), (xt[:, ct, :]), start=st, stop=sp
                        )
                        nc.tensor.matmul(
                            pk, (wk_sb[:, ct, :]), (xt[:, ct, :]), start=st, stop=sp
                        )
                        nc.tensor.matmul(
                            pv, (wv_sb[:, ct, :]), (xt[:, ct, :]), start=st, stop=sp
                        )
                    nc.vector.tensor_scalar_add(qt_s[:, t0:t0 + 512], pq, bq_sb)
                    nc.vector.tensor_scalar_add(kt_s[:, t0:t0 + 512], pk, bk_sb)
                    vt = vtt.tile([128, 512], F16, tag="vt")
                    nc.vector.tensor_scalar_add(vt, pv, bv_sb)
                    # transpose V into token-major vtm blocks
                    for j in range(4):
                        blk = tt * 4 + j
                        ptp = ps1.tile([128, 128], F16, tag="tp", bufs=2)
                        nc.tensor.transpose(
                            ptp, vt[:, j * 128:(j + 1) * 128], ident
                        )
                        nc.vector.tensor_copy(
                            vtm[:, :, blk, 0:64],
                            ptp.rearrange("p (h d) -> p h d", h=H_LOC),
                        )

            # ============ phase 2+3: attention + out projection ===========
            # PSUM budget (8 banks x 2KB/partition):
            #   "st"  [128,2,512]f32 = 2 banks x bufs=2  -> 4
            #   "av"  [128,512]f32   = 1 bank  x bufs=2  -> 2
            #   "lb"  [128,512]f32   = 1 bank  x bufs=2  -> 2 (shared by the
            #         1/l broadcasts and the out-proj accumulators)
            with (
                tc.tile_pool(name="ptp", bufs=3) as ptpool,
                tc.tile_pool(name="m2", bufs=2) as m2,
                tc.tile_pool(name="ob", bufs=3) as obp,
                tc.tile_pool(name="ps2", bufs=1, space="PSUM") as ps2,
            ):

                def emit_proj(g0, uid):
                    # partial out-projection for the 512-token chunk at g0;
                    # deferred one chunk so its ot_s dependencies are ready
                    # by the time the PE reaches these matmuls.
                    for ti in range(4):
                        t0 = g0 + ti * 128
                        ob = obp.tile(
                            [128, 2, 512], F16, tag="ob", name=f"ob_{uid}_{ti}",
                        )
                        for co in range(2):
                            po = ps2.tile(
                                [128, 512], F32, tag="po", bufs=2,
                                name=f"po_{uid}_{ti}_{co}",
                            )
                            nc.tensor.matmul(
                                po,
                                (ot_s[:, t0:t0 + 128]),
                                (wp_sb[:, co * 512:(co + 1) * 512]),
                                start=True, stop=True,
                            )
                            # split PSUM evacuation across DVE and Scalar
                            if co == 0:
                                nc.vector.tensor_copy(ob[:, co, :], po)
                            else:
                                nc.scalar.copy(ob[:, co, :], po)
                        nc.sync.dma_start(out=outp[t0:t0 + 128, :], in_=ob)

                pending_proj = None
                for b in range(B):
                    for w in range(NW):
                        g0 = b * T + w * 512      # global tok of chunk start
                        nk = (w + 1) * 4          # k-tiles of 128 in play
                        avs = {}
                        for h in range(H_LOC):
                            avs[h] = ps2.tile(
                                [128, 512], F32, tag="av", bufs=2,
                                name=f"av_{b}_{w}_{h}",
                            )
                        # software-pipelined ki loop: S(ki+1) is emitted
                        # before AV(ki) so the PE has work while Exp runs.
                        stts = {}
                        ptts = {}

                        def emit_s(ki):
                            off = max(0, ki * 128 - w * 512)
                            kg = b * T + ki * 128
                            stt = ps2.tile(
                                [128, 2, 512], F32, tag="st", bufs=2,
                                name=f"st_{b}_{w}_{ki}",
                            )
                            for h in range(H_LOC):
                                hd = h * 64
                                nc.tensor.matmul(
                                    stt[:, h, off:512],
                                    (kt_s[hd:hd + 64, kg:kg + 128]),
                                    (qt_s[hd:hd + 64, g0 + off:g0 + 512]),
                                    start=True, stop=True,
                                )
                            stts[ki] = stt

                        def emit_exp(ki):
                            off = max(0, ki * 128 - w * 512)
                            stt = stts.pop(ki)
                            ptt = ptpool.tile(
                                [128, 2, 512], F16, tag="pt",
                                name=f"pt_{b}_{w}_{ki}",
                            )
                            nc.scalar.activation(
                                ptt[:, :, off:512], stt[:, :, off:512],
                                AF.Exp, scale=SCALE,
                            )
                            if ki >= 4 * w:
                                # diagonal block: zero P where q < k (below
                                # causal diagonal), on the idle Pool engine
                                for h in range(H_LOC):
                                    nc.gpsimd.affine_select(
                                        out=ptt[:, h, off:off + 128],
                                        in_=ptt[:, h, off:off + 128],
                                        compare_op=ALU.is_ge,
                                        fill=0.0,
                                        base=0,
                                        pattern=[[1, 128]],
                                        channel_multiplier=-1,
                                    )  # keeps P where (q - k) >= 0
                            ptts[ki] = ptt

                        def emit_av(ki):
                            off = max(0, ki * 128 - w * 512)
                            ptt = ptts.pop(ki)
                            for h in range(H_LOC):
                                nc.tensor.matmul(
                                    avs[h][0:65, off:512],
                                    (vtm[:, h, (b * T) // 128 + ki, :]),
                                    (ptt[:, h, off:512]),
                                    start=(ki == 0),
                                    stop=(ki == nk - 1),
                                )

                        emit_s(0)
                        emit_exp(0)
                        if nk > 1:
                            emit_s(1)
                            emit_exp(1)
                        # previous chunk's out-projection: its ot_s inputs
                        # finished while the S tiles above were computed.
                        if pending_proj is not None:
                            emit_proj(*pending_proj)
                            pending_proj = None
                        for ki in range(nk):
                            if ki + 2 < nk:
                                emit_s(ki + 2)
                                emit_exp(ki + 2)
                            emit_av(ki)

                        # --- normalize by l -> ot_s (d-major packed) ------
                        # l_h sits in PSUM row 64 of avs[h]: copy to SBUF,
                        # broadcast across 64 partitions with a K=1
                        # ones-matmul, fast-reciprocal, multiply on DVE.
                        l_sb = m2.tile([65, 2, 512], F32R, tag="l_sb")
                        linv = m2.tile([64, 2, 512], F32, tag="linv")
                        for h in range(H_LOC):
                            nc.vector.tensor_copy(
                                l_sb[64:65, h, :], avs[h][64:65, :]
                            )
                            lb = ps2.tile(
                                [128, 512], F32, tag="po", bufs=2,
                                name=f"lb_{b}_{w}_{h}",
                            )
                            nc.tensor.matmul(
                                lb[0:64, :],
                                ones_r[64:65, 0:64],
                                l_sb[64:65, h, :],
                                start=True, stop=True,
                            )
                            nc.vector.reciprocal_approx_fast(
                                linv[:, h, :], lb[0:64, :]
                            )
                        stg = m2.tile([64, 512], F16, tag="stg")
                        nc.vector.tensor_mul(
                            ot_s[0:64, g0:g0 + 512], avs[0][0:64, :],
                            linv[:, 0, :],
                        )
                        nc.vector.tensor_mul(
                            stg, avs[1][0:64, :], linv[:, 1, :]
                        )
                        nc.sync.dma_start(
                            out=ot_s[64:128, g0:g0 + 512], in_=stg
                        )
                        pending_proj = (g0, f"{b}_{w}")
                emit_proj(*pending_proj)
    nc.compile()
    return nc


_PROGRAM = None


def _get_program():
    global _PROGRAM
    if _PROGRAM is None:
        _PROGRAM = build_program()
    return _PROGRAM


def _make_in_maps(x, W_qkv, b_qkv, W_proj):
    B, T, C = x.shape
    xT = np.ascontiguousarray(
        x.reshape(B * T, C).T.astype(np.float16)
    )
    in_maps = []
    for c in range(N_CORES):
        lo, hi = c * D_LOC, (c + 1) * D_LOC
        in_maps.append({
            "xT": xT,
            "wq": np.ascontiguousarray(W_qkv[:, lo:hi], np.float16),
            "wk": np.ascontiguousarray(W_qkv[:, C + lo:C + hi], np.float16),
            "wv": np.ascontiguousarray(W_qkv[:, 2 * C + lo:2 * C + hi], np.float16),
            "bq": np.ascontiguousarray(b_qkv[lo:hi].reshape(-1, 1), np.float32),
            "bk": np.ascontiguousarray(b_qkv[C + lo:C + hi].reshape(-1, 1), np.float32),
            "bv": np.ascontiguousarray(b_qkv[2 * C + lo:2 * C + hi].reshape(-1, 1), np.float32),
            "wp": np.ascontiguousarray(W_proj[lo:hi, :], np.float16),
        })
    return in_maps


LAST_RESULT = None


def run(inputs, trace=False):
    """Returns (full output [B,T,C] float32, exec_time_ns or None)."""
    global LAST_RESULT
    x = np.asarray(inputs["x"], np.float32)
    W_qkv = np.asarray(inputs["W_qkv"], np.float32)
    b_qkv = np.asarray(inputs["b_qkv"], np.float32)
    W_proj = np.asarray(inputs["W_proj"], np.float32)
    b_proj = np.asarray(inputs["b_proj"], np.float32)
    B, T, C = x.shape

    nc = _get_program()
    in_maps = _make_in_maps(x, W_qkv, b_qkv, W_proj)
    res = run_bass_kernel_spmd(
        nc, in_maps, list(range(N_CORES)), trace=trace
    )
    LAST_RESULT = res
    acc = np.zeros((B * T, C), np.float64)
    for c in range(N_CORES):
        acc += res.results[c]["outp"].astype(np.float64)
    out = (acc + b_proj.astype(np.float64)).astype(np.float32)
    return out.reshape(B, T, C), res.exec_time_ns


def kernel(**inputs):
    out, _ = run(inputs, trace=False)
    return out


# revision 33
# speedup vs baseline: 1.0439x; 1.0439x over previous
"""Causal self-attention (B=4, T=2048, C=1024, H=16) on 8 trn2 NeuronCores.

Sharding: tensor-parallel over heads. Core c owns heads {2c, 2c+1}:
  - computes Q,K,V projections for its 2 heads (full batch/sequence),
  - causal attention for its heads,
  - a partial output projection (row-slice of W_proj),
and the host sums the 8 partial projections (+ b_proj).

v2 changes vs baseline (705us):
  - All matmul operands in fp16 instead of float32r. fp32-HIGH matmuls
    trip the TRN2 power throttle (HAM shows util limited to 4/8 for 79%
    of the run); fp16 runs 1 cyc/row at full 2.4 GHz without throttling
    and is precise enough (rel err ~1e-3 << 2e-2 gate).
  - Attention processed in 512-wide q chunks; the two heads' S^T tiles
    live in one [128, 2, 512] PSUM tile so a single Exp activation
    covers both heads (halves scalar-engine instruction overhead).
  - Softmax denominators: reciprocal_approx_fast on the [1,512] l-row
    (the old full-tile nc.vector.reciprocal cost 106us of DVE time).
  - PSUM->SBUF evacuations (out-proj tiles, V transposes) and the
    causal mask add run on the idle Pool/GpSimd engine instead of DVE.
  - Output partials stored as fp16 (halves output DMA), summed on host.
  - x loaded with one DMA per 512-token tile instead of 8.
"""

import numpy as np

import concourse.bacc as bacc
import concourse.bass as bass
import concourse.tile as tile
from concourse import mybir
from concourse.bass_utils import run_bass_kernel_spmd
from concourse.masks import make_identity

F32 = mybir.dt.float32
F32R = mybir.dt.float32r
F16 = mybir.dt.float16
AF = mybir.ActivationFunctionType
ALU = mybir.AluOpType

N_CORES = 8
D_MODEL = 1024
N_HEADS = 16
HEAD_DIM = 64
H_LOC = 2            # heads per core
D_LOC = H_LOC * HEAD_DIM   # 128
SCALE = 1.0 / np.sqrt(HEAD_DIM)
NEG = -30000.0       # masked logits; exp(scale*NEG) == 0, fp16-safe


def build_program(B=4, T=2048):
    TOK = B * T
    TT = TOK // 512          # tok tiles of 512 for the QKV matmul
    CT = D_MODEL // 128      # contraction tiles
    NW = T // 512            # q-chunks per batch (512 wide)
    assert T % 512 == 0 and TOK % 512 == 0

    nc = bacc.Bacc(
        "TRN2", target_bir_lowering=False, debug=False, num_devices=N_CORES
    )
    xT = nc.dram_tensor("xT", [D_MODEL, TOK], F16, kind="ExternalInput").ap()
    wq = nc.dram_tensor("wq", [D_MODEL, D_LOC], F16, kind="ExternalInput").ap()
    wk = nc.dram_tensor("wk", [D_MODEL, D_LOC], F16, kind="ExternalInput").ap()
    wv = nc.dram_tensor("wv", [D_MODEL, D_LOC], F16, kind="ExternalInput").ap()
    bq = nc.dram_tensor("bq", [D_LOC, 1], F32, kind="ExternalInput").ap()
    bk = nc.dram_tensor("bk", [D_LOC, 1], F32, kind="ExternalInput").ap()
    bv = nc.dram_tensor("bv", [D_LOC, 1], F32, kind="ExternalInput").ap()
    wp = nc.dram_tensor("wp", [D_LOC, D_MODEL], F16, kind="ExternalInput").ap()
    outp = nc.dram_tensor("outp", [TOK, D_MODEL], F16, kind="ExternalOutput").ap()

    with tile.TileContext(nc) as tc:
        with (
            tc.tile_pool(name="const", bufs=1) as const,
            tc.tile_pool(name="res", bufs=1) as res,
        ):
            # --- constants -------------------------------------------------
            wq_sb = const.tile([128, CT, D_LOC], F16, tag="wq")
            wk_sb = const.tile([128, CT, D_LOC], F16, tag="wk")
            wv_sb = const.tile([128, CT, D_LOC], F16, tag="wv")
            for w_sb, w_dram in ((wq_sb, wq), (wk_sb, wk), (wv_sb, wv)):
                nc.sync.dma_start(
                    out=w_sb, in_=w_dram.rearrange("(ct p) d -> p ct d", p=128)
                )
            wp_sb = const.tile([128, D_MODEL], F16, tag="wp")
            nc.sync.dma_start(out=wp_sb, in_=wp)
            bq_sb = const.tile([128, 1], F32, tag="bq")
            bk_sb = const.tile([128, 1], F32, tag="bk")
            bv_sb = const.tile([128, 1], F32, tag="bv")
            for b_sb, b_dram in ((bq_sb, bq), (bk_sb, bk), (bv_sb, bv)):
                nc.sync.dma_start(out=b_sb, in_=b_dram)

            ones_f32 = const.tile([128, 128], F32, tag="ones_f32")
            nc.vector.memset(ones_f32, 1.0)
            ones_r = const.tile([128, 128], F32R, tag="ones_r")
            nc.vector.tensor_copy(ones_r, ones_f32)
            ident_f32 = const.tile([128, 128], F32, tag="ident_f32")
            make_identity(nc, ident_f32)
            ident = const.tile([128, 128], F16, tag="ident")
            nc.vector.tensor_copy(ident, ident_f32)

            # --- resident tensors -----------------------------------------
            qt_s = res.tile([128, TOK], F16, tag="qt")   # [d(2 heads), tok]
            kt_s = res.tile([128, TOK], F16, tag="kt")
            # token-major V with ones column: [tok(128), head, blk, 65]
            vtm = res.tile([128, H_LOC, TOK // 128, 65], F16, tag="vtm")
            ot_s = res.tile([128, TOK], F16, tag="ot")   # attention out, d-major
            nc.vector.tensor_copy(
                vtm[:, :, :, 64],
                ones_f32.rearrange("p (h b) -> p h b", h=H_LOC)[:, :, :TOK // 128],
            )

            # ================= phase 1: QKV projections ===================
            with (
                tc.tile_pool(name="xst", bufs=3) as xst,
                tc.tile_pool(name="vtt", bufs=3) as vtt,
                tc.tile_pool(name="ps1", bufs=1, space="PSUM") as ps1,
            ):
                for tt in range(TT):
                    t0 = tt * 512
                    xt = xst.tile([128, CT, 512], F16, tag="x")
                    # issue x loads on the (idle in phase 1) Activation HWDGE
                    # queue so they run in parallel with the weight loads on
                    # the SP queue.
                    nc.scalar.dma_start(
                        out=xt,
                        in_=xT.rearrange("(ct p) t -> p ct t", p=128)[
                            :, :, t0:t0 + 512
                        ],
                    )
                    pq = ps1.tile([128, 512], F32, tag="acc", bufs=6)
                    pk = ps1.tile([128, 512], F32, tag="acc", bufs=6)
                    pv = ps1.tile([128, 512], F32, tag="acc", bufs=6)
                    for ct in range(CT):
                        st, sp = ct == 0, ct == CT - 1
                        nc.tensor.matmul(
                            pq, (wq_sb[:, ct, :]), (xt[:, ct, :]), start=st, stop=sp
                        )
                        nc.tensor.matmul(
                            pk, (wk_sb[:, ct, :]), (xt[:, ct, :]), start=st, stop=sp
                        )
                        nc.tensor.matmul(
                            pv, (wv_sb[:, ct, :]), (xt[:, ct, :]), start=st, stop=sp
                        )
                    nc.vector.tensor_scalar_add(qt_s[:, t0:t0 + 512], pq, bq_sb)
                    nc.vector.tensor_scalar_add(kt_s[:, t0:t0 + 512], pk, bk_sb)
                    vt = vtt.tile([128, 512], F16, tag="vt")
                    nc.vector.tensor_scalar_add(vt, pv, bv_sb)
                    # transpose V into token-major vtm blocks
                    for j in range(4):
                        blk = tt * 4 + j
                        ptp = ps1.tile([128, 128], F16, tag="tp", bufs=2)
                        nc.tensor.transpose(
                            ptp, vt[:, j * 128:(j + 1) * 128], ident
                        )
                        # scalar engine is otherwise idle in phase 1
                        nc.scalar.copy(
                            vtm[:, :, blk, 0:64],
                            ptp.rearrange("p (h d) -> p h d", h=H_LOC),
                        )

            # ============ phase 2+3: attention + out projection ===========
            # PSUM budget (8 banks x 2KB/partition):
            #   "st"  [128,2,512]f32 = 2 banks x bufs=2  -> 4
            #   "av"  [128,512]f32   = 1 bank  x bufs=2  -> 2
            #   "lb"  [128,512]f32   = 1 bank  x bufs=2  -> 2 (shared by the
            #         1/l broadcasts and the out-proj accumulators)
            with (
                tc.tile_pool(name="ptp", bufs=6) as ptpool,
                tc.tile_pool(name="m2", bufs=2) as m2,
                tc.tile_pool(name="ob", bufs=3) as obp,
                tc.tile_pool(name="ps2", bufs=1, space="PSUM") as ps2,
            ):

                def emit_proj(g0, uid, last=False):
                    # partial out-projection for the 512-token chunk at g0;
                    # deferred one chunk so its ot_s dependencies are ready
                    # by the time the PE reaches these matmuls.
                    for ti in range(4):
                        t0 = g0 + ti * 128
                        ob = obp.tile(
                            [128, 2, 512], F16, tag="ob", name=f"ob_{uid}_{ti}",
                        )
                        for co in range(2):
                            po = ps2.tile(
                                [128, 512], F32, tag="po", bufs=2,
                                name=f"po_{uid}_{ti}_{co}",
                            )
                            nc.tensor.matmul(
                                po,
                                (ot_s[:, t0:t0 + 128]),
                                (wp_sb[:, co * 512:(co + 1) * 512]),
                                start=True, stop=True,
                            )
                            # PSUM evacuation on DVE: the scalar queue is
                            # saturated by the exp chain in phase 2 and an
                            # interleaved copy would delay exp -> AV. In the
                            # epilogue (no exps left) split across both.
                            if last and co == 1:
                                nc.scalar.copy(ob[:, co, :], po)
                            else:
                                nc.vector.tensor_copy(ob[:, co, :], po)
                        nc.sync.dma_start(out=outp[t0:t0 + 128, :], in_=ob)

                pending_proj = None
                for b in range(B):
                    for w in range(NW):
                        g0 = b * T + w * 512      # global tok of chunk start
                        nk = (w + 1) * 4          # k-tiles of 128 in play
                        avs = {}
                        for h in range(H_LOC):
                            avs[h] = ps2.tile(
                                [128, 512], F32, tag="av", bufs=2,
                                name=f"av_{b}_{w}_{h}",
                            )
                        # software-pipelined ki loop: S(ki+1) is emitted
                        # before AV(ki) so the PE has work while Exp runs.
                        stts = {}
                        ptts = {}

                        def emit_s(ki):
                            off = max(0, ki * 128 - w * 512)
                            kg = b * T + ki * 128
                            stt = ps2.tile(
                                [128, 2, 512], F32, tag="st", bufs=2,
                                name=f"st_{b}_{w}_{ki}",
                            )
                            for h in range(H_LOC):
                                hd = h * 64
                                nc.tensor.matmul(
                                    stt[:, h, off:512],
                                    (kt_s[hd:hd + 64, kg:kg + 128]),
                                    (qt_s[hd:hd + 64, g0 + off:g0 + 512]),
                                    start=True, stop=True,
                                )
                            stts[ki] = stt

                        def emit_exp(ki):
                            off = max(0, ki * 128 - w * 512)
                            stt = stts.pop(ki)
                            ptt = ptpool.tile(
                                [128, 2, 512], F16, tag="pt",
                                name=f"pt_{b}_{w}_{ki}",
                            )
                            nc.scalar.activation(
                                ptt[:, :, off:512], stt[:, :, off:512],
                                AF.Exp, scale=SCALE,
                            )
                            if ki >= 4 * w:
                                # diagonal block: zero P where q < k (below
                                # causal diagonal), on the idle Pool engine
                                for h in range(H_LOC):
                                    nc.gpsimd.affine_select(
                                        out=ptt[:, h, off:off + 128],
                                        in_=ptt[:, h, off:off + 128],
                                        compare_op=ALU.is_ge,
                                        fill=0.0,
                                        base=0,
                                        pattern=[[1, 128]],
                                        channel_multiplier=-1,
                                    )  # keeps P where (q - k) >= 0
                            ptts[ki] = ptt

                        def emit_av(ki):
                            off = max(0, ki * 128 - w * 512)
                            ptt = ptts.pop(ki)
                            for h in range(H_LOC):
                                nc.tensor.matmul(
                                    avs[h][0:65, off:512],
                                    (vtm[:, h, (b * T) // 128 + ki, :]),
                                    (ptt[:, h, off:512]),
                                    start=(ki == 0),
                                    stop=(ki == nk - 1),
                                )

                        # S/exp prefetch: gives the PE a queue of independent
                        # S matmuls (and the previous chunk's out-projection)
                        # to chew on while the previous chunk's normalization
                        # chain completes, and decouples each AV from its exp.
                        # The "st" PSUM ring holds 2 tiles, so only 2 S's can
                        # run before exp(0) completes — the proj batch goes
                        # between them in the PE queue.
                        DEPTH = 4
                        for ki in range(min(2, nk)):
                            emit_s(ki)
                            emit_exp(ki)
                        if pending_proj is not None:
                            emit_proj(*pending_proj)
                            pending_proj = None
                        for ki in range(2, min(DEPTH, nk)):
                            emit_s(ki)
                            emit_exp(ki)
                        for ki in range(nk):
                            if ki + DEPTH < nk:
                                emit_s(ki + DEPTH)
                                emit_exp(ki + DEPTH)
                            emit_av(ki)

                        # --- normalize by l -> ot_s (d-major packed) ------
                        # l_h sits in PSUM row 64 of avs[h]: 1/l via fast
                        # approx into SBUF, broadcast across 64 partitions
                        # with a gpsimd partition_broadcast (idle engine),
                        # multiply on DVE.
                        l_sb = m2.tile([65, 2, 512], F32R, tag="l_sb")
                        linv = m2.tile([64, 2, 512], F32, tag="linv")
                        for h in range(H_LOC):
                            nc.vector.tensor_copy(
                                l_sb[64:65, h, :], avs[h][64:65, :]
                            )
                            lb = ps2.tile(
                                [128, 512], F32, tag="po", bufs=2,
                                name=f"lb_{b}_{w}_{h}",
                            )
                            nc.tensor.matmul(
                                lb[0:64, :],
                                ones_r[64:65, 0:64],
                                l_sb[64:65, h, :],
                                start=True, stop=True,
                            )
                            nc.vector.reciprocal_approx_fast(
                                linv[:, h, :], lb[0:64, :]
                            )
                        stg = m2.tile([64, 512], F16, tag="stg")
                        nc.vector.tensor_mul(
                            ot_s[0:64, g0:g0 + 512], avs[0][0:64, :],
                            linv[:, 0, :],
                        )
                        nc.vector.tensor_mul(
                            stg, avs[1][0:64, :], linv[:, 1, :]
                        )
                        nc.sync.dma_start(
                            out=ot_s[64:128, g0:g0 + 512], in_=stg
                        )
                        pending_proj = (g0, f"{b}_{w}")
                emit_proj(*pending_proj, last=True)
    nc.compile()
    return nc


_PROGRAM = None


def _get_program():
    global _PROGRAM
    if _PROGRAM is None:
        _PROGRAM = build_program()
    return _PROGRAM


def _make_in_maps(x, W_qkv, b_qkv, W_proj):
    B, T, C = x.shape
    xT = np.ascontiguousarray(
        x.reshape(B * T, C).T.astype(np.float16)
    )
    in_maps = []
    for c in range(N_CORES):
        lo, hi = c * D_LOC, (c + 1) * D_LOC
        in_maps.append({
            "xT": xT,
            "wq": np.ascontiguousarray(W_qkv[:, lo:hi], np.float16),
            "wk": np.ascontiguousarray(W_qkv[:, C + lo:C + hi], np.float16),
            "wv": np.ascontiguousarray(W_qkv[:, 2 * C + lo:2 * C + hi], np.float16),
            "bq": np.ascontiguousarray(b_qkv[lo:hi].reshape(-1, 1), np.float32),
            "bk": np.ascontiguousarray(b_qkv[C + lo:C + hi].reshape(-1, 1), np.float32),
            "bv": np.ascontiguousarray(b_qkv[2 * C + lo:2 * C + hi].reshape(-1, 1), np.float32),
            "wp": np.ascontiguousarray(W_proj[lo:hi, :], np.float16),
        })
    return in_maps


LAST_RESULT = None


def run(inputs, trace=False):
    """Returns (full output [B,T,C] float32, exec_time_ns or None)."""
    global LAST_RESULT
    x = np.asarray(inputs["x"], np.float32)
    W_qkv = np.asarray(inputs["W_qkv"], np.float32)
    b_qkv = np.asarray(inputs["b_qkv"], np.float32)
    W_proj = np.asarray(inputs["W_proj"], np.float32)
    b_proj = np.asarray(inputs["b_proj"], np.float32)
    B, T, C = x.shape

    nc = _get_program()
    in_maps = _make_in_maps(x, W_qkv, b_qkv, W_proj)
    res = run_bass_kernel_spmd(
        nc, in_maps, list(range(N_CORES)), trace=trace
    )
    LAST_RESULT = res
    acc = np.zeros((B * T, C), np.float64)
    for c in range(N_CORES):
        acc += res.results[c]["outp"].astype(np.float64)
    out = (acc + b_proj.astype(np.float64)).astype(np.float32)
    return out.reshape(B, T, C), res.exec_time_ns


def kernel(**inputs):
    out, _ = run(inputs, trace=False)
    return out


# revision 37
# speedup vs baseline: 1.0808x; 1.0354x over previous
"""Causal self-attention (B=4, T=2048, C=1024, H=16) on 8 trn2 NeuronCores.

Sharding: tensor-parallel over heads. Core c owns heads {2c, 2c+1}:
  - computes Q,K,V projections for its 2 heads (full batch/sequence),
  - causal attention for its heads,
  - a partial output projection (row-slice of W_proj),
and the host sums the 8 partial projections (+ b_proj).

v2 changes vs baseline (705us):
  - All matmul operands in fp16 instead of float32r. fp32-HIGH matmuls
    trip the TRN2 power throttle (HAM shows util limited to 4/8 for 79%
    of the run); fp16 runs 1 cyc/row at full 2.4 GHz without throttling
    and is precise enough (rel err ~1e-3 << 2e-2 gate).
  - Attention processed in 512-wide q chunks; the two heads' S^T tiles
    live in one [128, 2, 512] PSUM tile so a single Exp activation
    covers both heads (halves scalar-engine instruction overhead).
  - Softmax denominators: reciprocal_approx_fast on the [1,512] l-row
    (the old full-tile nc.vector.reciprocal cost 106us of DVE time).
  - PSUM->SBUF evacuations (out-proj tiles, V transposes) and the
    causal mask add run on the idle Pool/GpSimd engine instead of DVE.
  - Output partials stored as fp16 (halves output DMA), summed on host.
  - x loaded with one DMA per 512-token tile instead of 8.
"""

import numpy as np

import concourse.bacc as bacc
import concourse.bass as bass
import concourse.tile as tile
from concourse import mybir
from concourse.bass_utils import run_bass_kernel_spmd
from concourse.masks import make_identity

F32 = mybir.dt.float32
F32R = mybir.dt.float32r
F16 = mybir.dt.float16
AF = mybir.ActivationFunctionType
ALU = mybir.AluOpType

N_CORES = 8
D_MODEL = 1024
N_HEADS = 16
HEAD_DIM = 64
H_LOC = 2            # heads per core
D_LOC = H_LOC * HEAD_DIM   # 128
SCALE = 1.0 / np.sqrt(HEAD_DIM)
NEG = -30000.0       # masked logits; exp(scale*NEG) == 0, fp16-safe


def build_program(B=4, T=2048):
    TOK = B * T
    TT = TOK // 512          # tok tiles of 512 for the QKV matmul
    CT = D_MODEL // 128      # contraction tiles
    NW = T // 512            # q-chunks per batch (512 wide)
    assert T % 512 == 0 and TOK % 512 == 0

    nc = bacc.Bacc(
        "TRN2", target_bir_lowering=False, debug=False, num_devices=N_CORES
    )
    xT = nc.dram_tensor("xT", [D_MODEL, TOK], F16, kind="ExternalInput").ap()
    wq = nc.dram_tensor("wq", [D_MODEL, D_LOC], F16, kind="ExternalInput").ap()
    wk = nc.dram_tensor("wk", [D_MODEL, D_LOC], F16, kind="ExternalInput").ap()
    wv = nc.dram_tensor("wv", [D_MODEL, D_LOC], F16, kind="ExternalInput").ap()
    bq = nc.dram_tensor("bq", [D_LOC, 1], F32, kind="ExternalInput").ap()
    bk = nc.dram_tensor("bk", [D_LOC, 1], F32, kind="ExternalInput").ap()
    bv = nc.dram_tensor("bv", [D_LOC, 1], F32, kind="ExternalInput").ap()
    wp = nc.dram_tensor("wp", [D_LOC, D_MODEL], F16, kind="ExternalInput").ap()
    outp = nc.dram_tensor("outp", [TOK, D_MODEL], F16, kind="ExternalOutput").ap()

    with tile.TileContext(nc) as tc:
        with (
            tc.tile_pool(name="const", bufs=1) as const,
            tc.tile_pool(name="res", bufs=1) as res,
        ):
            # --- constants -------------------------------------------------
            wq_sb = const.tile([128, CT, D_LOC], F16, tag="wq")
            wk_sb = const.tile([128, CT, D_LOC], F16, tag="wk")
            wv_sb = const.tile([128, CT, D_LOC], F16, tag="wv")
            for w_sb, w_dram in ((wq_sb, wq), (wk_sb, wk), (wv_sb, wv)):
                nc.sync.dma_start(
                    out=w_sb, in_=w_dram.rearrange("(ct p) d -> p ct d", p=128)
                )
            wp_sb = const.tile([128, D_MODEL], F16, tag="wp")
            nc.sync.dma_start(out=wp_sb, in_=wp)
            bq_sb = const.tile([128, 1], F32, tag="bq")
            bk_sb = const.tile([128, 1], F32, tag="bk")
            bv_sb = const.tile([128, 1], F32, tag="bv")
            for b_sb, b_dram in ((bq_sb, bq), (bk_sb, bk), (bv_sb, bv)):
                nc.sync.dma_start(out=b_sb, in_=b_dram)

            ones_f32 = const.tile([128, 128], F32, tag="ones_f32")
            nc.vector.memset(ones_f32, 1.0)
            ones_r = const.tile([128, 128], F32R, tag="ones_r")
            nc.vector.tensor_copy(ones_r, ones_f32)
            ident_f32 = const.tile([128, 128], F32, tag="ident_f32")
            make_identity(nc, ident_f32)
            ident = const.tile([128, 128], F16, tag="ident")
            nc.vector.tensor_copy(ident, ident_f32)

            # --- resident tensors -----------------------------------------
            qt_s = res.tile([128, TOK], F16, tag="qt")   # [d(2 heads), tok]
            kt_s = res.tile([128, TOK], F16, tag="kt")
            # token-major V with ones column: [tok(128), head, blk, 65]
            vtm = res.tile([128, H_LOC, TOK // 128, 65], F16, tag="vtm")
            ot_s = res.tile([128, TOK], F16, tag="ot")   # attention out, d-major
            nc.vector.tensor_copy(
                vtm[:, :, :, 64],
                ones_f32.rearrange("p (h b) -> p h b", h=H_LOC)[:, :, :TOK // 128],
            )

            # ================= phase 1: QKV projections ===================
            with (
                tc.tile_pool(name="xst", bufs=3) as xst,
                tc.tile_pool(name="vtt", bufs=3) as vtt,
                tc.tile_pool(name="ps1", bufs=1, space="PSUM") as ps1,
            ):
                xTr = xT.rearrange("(ct p) t -> p ct t", p=128)
                for tt in range(TT):
                    t0 = tt * 512
                    xt = xst.tile([128, CT, 512], F16, tag="x")
                    # issue x loads on the (idle in phase 1) Activation HWDGE
                    # queue so they run in parallel with the weight loads on
                    # the SP queue. The first tile is split per contraction
                    # slice across both queues so the first matmul can start
                    # as early as possible.
                    if tt == 0:
                        for ct in range(CT):
                            eng = nc.scalar if ct % 2 == 0 else nc.sync
                            eng.dma_start(
                                out=xt[:, ct, :],
                                in_=xTr[:, ct, t0:t0 + 512],
                            )
                    else:
                        nc.scalar.dma_start(
                            out=xt, in_=xTr[:, :, t0:t0 + 512]
                        )
                    pq = ps1.tile([128, 512], F32, tag="acc", bufs=6)
                    pk = ps1.tile([128, 512], F32, tag="acc", bufs=6)
                    pv = ps1.tile([128, 512], F32, tag="acc", bufs=6)
                    for ct in range(CT):
                        st, sp = ct == 0, ct == CT - 1
                        nc.tensor.matmul(
                            pq, (wq_sb[:, ct, :]), (xt[:, ct, :]), start=st, stop=sp
                        )
                        nc.tensor.matmul(
                            pk, (wk_sb[:, ct, :]), (xt[:, ct, :]), start=st, stop=sp
                        )
                        nc.tensor.matmul(
                            pv, (wv_sb[:, ct, :]), (xt[:, ct, :]), start=st, stop=sp
                        )
                    nc.vector.tensor_scalar_add(qt_s[:, t0:t0 + 512], pq, bq_sb)
                    nc.vector.tensor_scalar_add(kt_s[:, t0:t0 + 512], pk, bk_sb)
                    vt = vtt.tile([128, 512], F16, tag="vt")
                    nc.vector.tensor_scalar_add(vt, pv, bv_sb)
                    # transpose V into token-major vtm blocks
                    for j in range(4):
                        blk = tt * 4 + j
                        ptp = ps1.tile([128, 128], F16, tag="tp", bufs=2)
                        nc.tensor.transpose(
                            ptp, vt[:, j * 128:(j + 1) * 128], ident
                        )
                        # scalar engine is otherwise idle in phase 1
                        nc.scalar.copy(
                            vtm[:, :, blk, 0:64],
                            ptp.rearrange("p (h d) -> p h d", h=H_LOC),
                        )

            # ============ phase 2+3: attention + out projection ===========
            # PSUM budget (8 banks x 2KB/partition):
            #   "st"  [128,2,512]f32 = 2 banks x bufs=2  -> 4
            #   "av"  [128,512]f32   = 1 bank  x bufs=2  -> 2
            #   "lb"  [128,512]f32   = 1 bank  x bufs=2  -> 2 (shared by the
            #         1/l broadcasts and the out-proj accumulators)
            with (
                tc.tile_pool(name="ptp", bufs=6) as ptpool,
                tc.tile_pool(name="m2", bufs=2) as m2,
                tc.tile_pool(name="ob", bufs=3) as obp,
                tc.tile_pool(name="ps2", bufs=1, space="PSUM") as ps2,
            ):

                def emit_proj(g0, uid, last=False):
                    # partial out-projection for the 512-token chunk at g0;
                    # deferred one chunk so its ot_s dependencies are ready
                    # by the time the PE reaches these matmuls.
                    for ti in range(4):
                        t0 = g0 + ti * 128
                        ob = obp.tile(
                            [128, 2, 512], F16, tag="ob", name=f"ob_{uid}_{ti}",
                        )
                        for co in range(2):
                            po = ps2.tile(
                                [128, 512], F32, tag="po", bufs=2,
                                name=f"po_{uid}_{ti}_{co}",
                            )
                            nc.tensor.matmul(
                                po,
                                (ot_s[:, t0:t0 + 128]),
                                (wp_sb[:, co * 512:(co + 1) * 512]),
                                start=True, stop=True,
                            )
                            # PSUM evacuation on DVE: the scalar queue is
                            # saturated by the exp chain in phase 2 and an
                            # interleaved copy would delay exp -> AV. In the
                            # epilogue (no exps left) split across both.
                            if last and co == 1:
                                nc.scalar.copy(ob[:, co, :], po)
                            else:
                                nc.vector.tensor_copy(ob[:, co, :], po)
                        nc.sync.dma_start(out=outp[t0:t0 + 128, :], in_=ob)

                def emit_norm(avs, g0, uid):
                    # normalize by l -> ot_s (d-major packed). l_h sits in
                    # PSUM row 64 of avs[h]: copy to SBUF, broadcast across
                    # 64 partitions with a K=1 ones-matmul, fast-reciprocal,
                    # multiply on DVE.  Deferred one chunk so the l_sb-copy
                    # latency hides behind the next chunk's S prefetch.
                    l_sb = m2.tile([65, 2, 512], F32R, tag="l_sb")
                    linv = m2.tile([64, 2, 512], F32, tag="linv")
                    for h in range(H_LOC):
                        nc.vector.tensor_copy(
                            l_sb[64:65, h, :], avs[h][64:65, :]
                        )
                        lb = ps2.tile(
                            [128, 512], F32, tag="po", bufs=2,
                            name=f"lb_{uid}_{h}",
                        )
                        nc.tensor.matmul(
                            lb[0:64, :],
                            ones_r[64:65, 0:64],
                            l_sb[64:65, h, :],
                            start=True, stop=True,
                        )
                        nc.vector.reciprocal_approx_fast(
                            linv[:, h, :], lb[0:64, :]
                        )
                    stg = m2.tile([64, 512], F16, tag="stg")
                    nc.vector.tensor_mul(
                        ot_s[0:64, g0:g0 + 512], avs[0][0:64, :],
                        linv[:, 0, :],
                    )
                    nc.vector.tensor_mul(
                        stg, avs[1][0:64, :], linv[:, 1, :]
                    )
                    nc.sync.dma_start(
                        out=ot_s[64:128, g0:g0 + 512], in_=stg
                    )

                pending_norm = None
                pending_proj = None
                for b in range(B):
                    for w in range(NW):
                        g0 = b * T + w * 512      # global tok of chunk start
                        nk = (w + 1) * 4          # k-tiles of 128 in play
                        avs = {}
                        for h in range(H_LOC):
                            avs[h] = ps2.tile(
                                [128, 512], F32, tag="av", bufs=2,
                                name=f"av_{b}_{w}_{h}",
                            )
                        # software-pipelined ki loop: S(ki+1) is emitted
                        # before AV(ki) so the PE has work while Exp runs.
                        stts = {}
                        ptts = {}

                        def emit_s(ki):
                            off = max(0, ki * 128 - w * 512)
                            kg = b * T + ki * 128
                            stt = ps2.tile(
                                [128, 2, 512], F32, tag="st", bufs=2,
                                name=f"st_{b}_{w}_{ki}",
                            )
                            for h in range(H_LOC):
                                hd = h * 64
                                nc.tensor.matmul(
                                    stt[:, h, off:512],
                                    (kt_s[hd:hd + 64, kg:kg + 128]),
                                    (qt_s[hd:hd + 64, g0 + off:g0 + 512]),
                                    start=True, stop=True,
                                )
                            stts[ki] = stt

                        def emit_exp(ki):
                            off = max(0, ki * 128 - w * 512)
                            stt = stts.pop(ki)
                            ptt = ptpool.tile(
                                [128, 2, 512], F16, tag="pt",
                                name=f"pt_{b}_{w}_{ki}",
                            )
                            nc.scalar.activation(
                                ptt[:, :, off:512], stt[:, :, off:512],
                                AF.Exp, scale=SCALE,
                            )
                            if ki >= 4 * w:
                                # diagonal block: zero P where q < k (below
                                # causal diagonal), on the idle Pool engine
                                for h in range(H_LOC):
                                    nc.gpsimd.affine_select(
                                        out=ptt[:, h, off:off + 128],
                                        in_=ptt[:, h, off:off + 128],
                                        compare_op=ALU.is_ge,
                                        fill=0.0,
                                        base=0,
                                        pattern=[[1, 128]],
                                        channel_multiplier=-1,
                                    )  # keeps P where (q - k) >= 0
                            ptts[ki] = ptt

                        def emit_av(ki):
                            off = max(0, ki * 128 - w * 512)
                            ptt = ptts.pop(ki)
                            for h in range(H_LOC):
                                nc.tensor.matmul(
                                    avs[h][0:65, off:512],
                                    (vtm[:, h, (b * T) // 128 + ki, :]),
                                    (ptt[:, h, off:512]),
                                    start=(ki == 0),
                                    stop=(ki == nk - 1),
                                )

                        # S/exp prefetch: gives the PE a queue of independent
                        # S matmuls (and the previous chunk's out-projection)
                        # to chew on while the previous chunk's normalization
                        # chain completes, and decouples each AV from its exp.
                        # The "st" PSUM ring holds 2 tiles, so only 2 S's can
                        # run before exp(0) completes — the proj batch goes
                        # between them in the PE queue.
                        DEPTH = 4
                        for ki in range(min(2, nk)):
                            emit_s(ki)
                            emit_exp(ki)
                        # chunk c-2's out-projection (its normalize ran a
                        # whole chunk ago) and chunk c-1's normalization
                        # slot in behind the S prefetch, keeping the PE fed
                        # across the chunk boundary.
                        if pending_proj is not None:
                            emit_proj(*pending_proj)
                            pending_proj = None
                        if pending_norm is not None:
                            navs, ng0, nuid = pending_norm
                            emit_norm(navs, ng0, nuid)
                            pending_proj = (ng0, nuid)
                            pending_norm = None
                        for ki in range(2, min(DEPTH, nk)):
                            emit_s(ki)
                            emit_exp(ki)
                        for ki in range(nk):
                            if ki + DEPTH < nk:
                                emit_s(ki + DEPTH)
                                emit_exp(ki + DEPTH)
                            emit_av(ki)

                        pending_norm = (avs, g0, f"{b}_{w}")
                if pending_proj is not None:
                    emit_proj(*pending_proj)
                navs, ng0, nuid = pending_norm
                emit_norm(navs, ng0, nuid)
                emit_proj(ng0, nuid, last=True)
    nc.compile()
    return nc


_PROGRAM = None


def _get_program():
    global _PROGRAM
    if _PROGRAM is None:
        _PROGRAM = build_program()
    return _PROGRAM


def _make_in_maps(x, W_qkv, b_qkv, W_proj):
    B, T, C = x.shape
    xT = np.ascontiguousarray(
        x.reshape(B * T, C).T.astype(np.float16)
    )
    in_maps = []
    for c in range(N_CORES):
        lo, hi = c * D_LOC, (c + 1) * D_LOC
        in_maps.append({
            "xT": xT,
            "wq": np.ascontiguousarray(W_qkv[:, lo:hi], np.float16),
            "wk": np.ascontiguousarray(W_qkv[:, C + lo:C + hi], np.float16),
            "wv": np.ascontiguousarray(W_qkv[:, 2 * C + lo:2 * C + hi], np.float16),
            "bq": np.ascontiguousarray(b_qkv[lo:hi].reshape(-1, 1), np.float32),
            "bk": np.ascontiguousarray(b_qkv[C + lo:C + hi].reshape(-1, 1), np.float32),
            "bv": np.ascontiguousarray(b_qkv[2 * C + lo:2 * C + hi].reshape(-1, 1), np.float32),
            "wp": np.ascontiguousarray(W_proj[lo:hi, :], np.float16),
        })
    return in_maps


LAST_RESULT = None


def run(inputs, trace=False):
    """Returns (full output [B,T,C] float32, exec_time_ns or None)."""
    global LAST_RESULT
    x = np.asarray(inputs["x"], np.float32)
    W_qkv = np.asarray(inputs["W_qkv"], np.float32)
    b_qkv = np.asarray(inputs["b_qkv"], np.float32)
    W_proj = np.asarray(inputs["W_proj"], np.float32)
    b_proj = np.asarray(inputs["b_proj"], np.float32)
    B, T, C = x.shape

    nc = _get_program()
    in_maps = _make_in_maps(x, W_qkv, b_qkv, W_proj)
    res = run_bass_kernel_spmd(
        nc, in_maps, list(range(N_CORES)), trace=trace
    )
    LAST_RESULT = res
    acc = np.zeros((B * T, C), np.float64)
    for c in range(N_CORES):
        acc += res.results[c]["outp"].astype(np.float64)
    out = (acc + b_proj.astype(np.float64)).astype(np.float32)
    return out.reshape(B, T, C), res.exec_time_ns


def kernel(**inputs):
    out, _ = run(inputs, trace=False)
    return out
